# revision 1
# baseline (speedup 1.0000x reference)
"""Trainium2 Bass kernel for DFlashAttentionV5.

Reference computation (fp32, single device):
    Q/K/V/Kctx/Vctx projections -> rmsnorm(Q), rmsnorm(K_full) -> softmax
    attention over concat(ctx, self) keys/values -> output projection.

Sharding over 8 NeuronCores: batch (2-way) x head-group (4-way).
Core c handles batch b = c // 4 and heads 4*g..4*g+3 where g = c % 4.
Each core computes attention for its 4 local heads; after each head it
all-gathers that head's attention output (transposed layout [hd, tokens])
across its 4-core batch group and immediately accumulates that head-wave
of the output projection for its 512-wide output column slice, so the
collectives and the output projection overlap the remaining heads.

All matmuls run in bf16 (fp32 PSUM accumulation); softmax statistics and
normalization factors are computed in fp32. x/ctx arrive pre-transposed
from the host ([D, tokens]) so no on-device transposes are needed.

Self-contained: hardcodes all shapes; only imports concourse + numpy.
"""

import math

import numpy as np
import ml_dtypes

import concourse.bass as bass
import concourse.mybir as mybir
import concourse.tile as tile
from concourse.bass_utils import run_bass_kernel_spmd

BF16 = mybir.dt.bfloat16
F32 = mybir.dt.float32
AF = mybir.ActivationFunctionType
ALU = mybir.AluOpType

# Problem dims
B, K, CTX, D, H, HD = 2, 1024, 2048, 2048, 16, 128
S = CTX + K            # 3072 keys per query
NCORES = 8
GROUPS = 4             # head groups (tensor-parallel within a batch)
NH = H // GROUPS       # 4 local heads per core
E = H * HD             # 2048
EW = NH * HD           # 512 local attention width / weight shard width
DCH = D // 128         # 16 contraction chunks
SCH = S // 128         # 24 key chunks
TCH = K // 128         # 8 query-token chunks
SCALE = 1.0 / math.sqrt(HD)
EPS = 1e-6
REPLICA_GROUPS = [[0, 1, 2, 3], [4, 5, 6, 7]]

_CACHE = {}


def _build(with_mask: bool):
    """Build the SPMD bass program (same program on all 8 cores)."""
    nc = bass.Bass(num_devices=NCORES)

    xT_d = nc.declare_dram_parameter("xT", [D, K], BF16, isOutput=False)
    cT_d = nc.declare_dram_parameter("cT", [D, CTX], BF16, isOutput=False)
    wq_d = nc.declare_dram_parameter("wq", [D, EW], BF16, isOutput=False)
    wk_d = nc.declare_dram_parameter("wk", [D, EW], BF16, isOutput=False)
    wv_d = nc.declare_dram_parameter("wv", [D, EW], BF16, isOutput=False)
    wck_d = nc.declare_dram_parameter("wck", [D, EW], BF16, isOutput=False)
    wcv_d = nc.declare_dram_parameter("wcv", [D, EW], BF16, isOutput=False)
    wo_d = nc.declare_dram_parameter("wo", [E, EW], BF16, isOutput=False)
    qnw_d = nc.declare_dram_parameter("qnw", [HD, 1], F32, isOutput=False)
    knw_d = nc.declare_dram_parameter("knw", [HD, 1], F32, isOutput=False)
    if with_mask:
        mt_d = nc.declare_dram_parameter("maskT", [S, K], F32, isOutput=False)
    out_d = nc.declare_dram_parameter("out", [K, EW], F32, isOutput=True)

    with tile.TileContext(nc, num_cores=NCORES) as tc:
        with (
            tc.tile_pool(name="const", bufs=1) as constp,
            tc.tile_pool(name="perm", bufs=1) as perm,
            tc.tile_pool(name="stat", bufs=2) as statp,
            tc.tile_pool(name="bc", bufs=2) as bcp,
            tc.tile_pool(name="psA", bufs=3, space="PSUM") as psA,
            tc.tile_pool(name="ps1", bufs=2, space="PSUM") as ps1,
            tc.tile_pool(name="dram", bufs=1, space="DRAM") as dram,
        ):
            ones_col = constp.tile([128, 1], BF16)
            nc.any.memset(ones_col, 1.0)
            ones_row = constp.tile([1, 128], BF16)
            nc.any.memset(ones_row, 1.0)
            qnw_sb = constp.tile([HD, 1], F32)
            knw_sb = constp.tile([HD, 1], F32)

            # Resident tensors (bf16):
            #   K_sb[h]  [128=hd, 3072=s] per local head (ctx keys then self)
            #   V_sb[s]  [128=s-chunk, 512=4 heads x hd], s 0..15 ctx, 16..23 self
            #   QT_sb[h] [128=hd, 1024=q]
            K_sb = [perm.tile([128, S], BF16, tag=f"K{h}", bufs=1, name=f"K{h}")
                    for h in range(NH)]
            V_sb = [perm.tile([128, EW], BF16, tag=f"V{s}", bufs=1, name=f"V{s}")
                    for s in range(SCH)]
            QT_sb = [perm.tile([128, K], BF16, tag=f"Q{h}", bufs=1, name=f"Q{h}")
                     for h in range(NH)]

            attnT_loc = [dram.tile([128, K], BF16, name=f"atl{h}")
                         for h in range(NH)]
            attnT_gat = [dram.tile([GROUPS * 128, K], BF16, name=f"atg{h}")
                         for h in range(NH)]

            # ---- helper: rmsnorm in transposed layout.
            # ps [128=hd, width=tokens] fp32 PSUM -> dest bf16 SBUF.
            # norm over hd (partitions): mean of squares via ones-matmul,
            # rsqrt via reciprocal+sqrt, broadcast across partitions via K=1
            # matmul, apply with one scalar_tensor_tensor (folds norm weight).
            def rms_norm_T(sqp, ps, dest_ap, width, nw_sb):
                # norm over hd (partitions): mean of squares via ones-matmul,
                # rsqrt via reciprocal+sqrt, broadcast across partitions via
                # K=1 matmul, applied with one scalar_tensor_tensor (also
                # folds the norm weight).
                sqt = sqp.tile([128, 1024], BF16, tag="sq")
                nc.scalar.square(sqt[:, :width], ps[:, :width])
                for j in range(width // 512):
                    js = slice(j * 512, (j + 1) * 512)
                    ps_s = ps1.tile([128, 512], F32, tag="ps1")
                    nc.tensor.matmul(ps_s[0:1, :], ones_col[:], sqt[:, js],
                                     start=True, stop=True)
                    mean = statp.tile([1, 512], F32, tag="mean")
                    nc.vector.tensor_scalar(mean[:], ps_s[0:1, :], 1.0 / HD, EPS,
                                            ALU.mult, ALU.add)
                    rec = statp.tile([1, 512], F32, tag="rec")
                    nc.vector.reciprocal(rec[:], mean[:])
                    rs = statp.tile([1, 512], BF16, tag="rs")
                    nc.scalar.sqrt(rs[:], rec[:])  # rsqrt = sqrt(1/x), bf16
                    ps_b = ps1.tile([128, 512], F32, tag="ps1")
                    nc.tensor.matmul(ps_b[:], ones_row[:], rs[:],
                                     start=True, stop=True)
                    bc = bcp.tile([128, 512], F32, tag="bc")
                    nc.scalar.copy(bc[:], ps_b[:])
                    nc.vector.scalar_tensor_tensor(
                        dest_ap[:, js], ps[:, js], nw_sb[:], bc[:],
                        ALU.mult, ALU.mult)

            # ================= projection phase =================
            with (
                tc.tile_pool(name="srcT", bufs=1) as srcTp,
                tc.tile_pool(name="wstream", bufs=4) as wstream,
                tc.tile_pool(name="wwide", bufs=1) as wwide,
                tc.tile_pool(name="sqp", bufs=2) as sqp,
            ):
                # d-chunk accessor over grouped source tiles [128, 4*1024]
                def src_at(grp, d):
                    return grp[d // 4], (d % 4) * 1024

                def load_set(grp, dram_ap, split_first=False):
                    # dram_ap: [D, 1024] (d-major); one DMA per 4 d-chunks.
                    # split_first peels d-chunk 0 into its own small DMA so
                    # the first dependent matmul can start sooner.
                    if split_first:
                        nc.sync.dma_start(
                            grp[0][:, 0:1024],
                            dram_ap[0:128, :])
                        nc.sync.dma_start(
                            grp[0][:, 1024:4096].rearrange("p (a t) -> p a t",
                                                           t=1024),
                            dram_ap[128:512, :]
                            .rearrange("(a p) t -> p a t", p=128))
                    else:
                        nc.sync.dma_start(
                            grp[0][:].rearrange("p (a t) -> p a t", t=1024),
                            dram_ap[0:512, :]
                            .rearrange("(a p) t -> p a t", p=128))
                    for i in range(1, 4):
                        nc.sync.dma_start(
                            grp[i][:].rearrange("p (a t) -> p a t", t=1024),
                            dram_ap[i * 512:(i + 1) * 512, :]
                            .rearrange("(a p) t -> p a t", p=128))

                # Q^T / K^T projections (weight-stationary):
                # psum[c] [128=col-chunk(head), 1024 tokens] += w[d,c].T @ srcT[d]
                def load_wchunk(w_d, c, name=None):
                    wch = wstream.tile([128, D], BF16, tag="w", name=name)
                    nc.sync.dma_start(
                        wch[:].rearrange("p (a q) -> p a q", q=128),
                        w_d[:, c * 128:(c + 1) * 128]
                        .rearrange("(a p) q -> p a q", p=128))
                    return wch

                def qk_proj(w_d, srcT, dest_of_chunk, nw_sb, pre=None):
                    for c in range(EW // 128):
                        wch = pre if (c == 0 and pre is not None) \
                            else load_wchunk(w_d, c)
                        ps = psA.tile([128, 1024], F32, tag="psA",
                                      name=f"psqk{c}")
                        for d in range(DCH):
                            st, off = src_at(srcT, d)
                            for j in range(2):
                                nc.tensor.matmul(
                                    ps[:, j * 512:(j + 1) * 512],
                                    wch[:, d * 128:(d + 1) * 128],
                                    st[:, off + j * 512:off + j * 512 + 512],
                                    start=(d == 0), stop=(d == DCH - 1))
                        dest, off = dest_of_chunk(c)
                        rms_norm_T(sqp, ps, dest[:, off:off + 1024], 1024, nw_sb)

                # V projections (activation-stationary):
                # V_sb[s] [128=tokens, 512=cols] += srcT[d][:,t-chunk].T @ wv[d]
                def load_wide(w_d, tag):
                    grp = []
                    for i in range(4):
                        wt = wwide.tile([128, 4 * EW], BF16, tag=f"wv{i}",
                                        bufs=1, name=f"wv{i}_{tag}")
                        nc.sync.dma_start(
                            wt[:].rearrange("p (a q) -> p a q", q=EW),
                            w_d[i * 512:(i + 1) * 512, :]
                            .rearrange("(a p) q -> p a q", p=128))
                        grp.append(wt)
                    return grp

                def v_proj(wv_grp, srcT, s_base):
                    for t in range(TCH):
                        ps = ps1.tile([128, 512], F32, tag="ps1")
                        for d in range(DCH):
                            st, off = src_at(srcT, d)
                            wvt = wv_grp[d // 4]
                            wo_off = (d % 4) * EW
                            nc.tensor.matmul(
                                ps[:], st[:, off + t * 128:off + (t + 1) * 128],
                                wvt[:, wo_off:wo_off + EW],
                                start=(d == 0), stop=(d == DCH - 1))
                        nc.vector.tensor_copy(V_sb[s_base + t][:], ps[:])

                # slot sets: A = xT then ctx-half1 (recycled), B = ctx-half0
                setA = [srcTp.tile([128, 4096], BF16, tag=f"sa{i}", bufs=1,
                                   name=f"xT{i}") for i in range(4)]
                setB = [srcTp.tile([128, 4096], BF16, tag=f"sb{i}", bufs=1,
                                   name=f"cTa{i}") for i in range(4)]
                pre_wq = load_wchunk(wq_d, 0, name="prewq")
                load_set(setA, xT_d, split_first=True)
                nc.sync.dma_start(qnw_sb[:], qnw_d[:])
                nc.sync.dma_start(knw_sb[:], knw_d[:])

                # self tokens (block A): Q, K_self, V_self
                qk_proj(wq_d, setA, lambda c: (QT_sb[c], 0), qnw_sb, pre=pre_wq)
                load_set(setB, cT_d[:, 0:1024])
                wv_sb = load_wide(wv_d, "s")
                qk_proj(wk_d, setA, lambda c: (K_sb[c], CTX), knw_sb)
                v_proj(wv_sb, setA, CTX // 128)

                # ctx half 0 (block B): K_ctx[:, 0:1024], V_ctx s-chunks 0..7
                wcv_sb = load_wide(wcv_d, "c0")
                qk_proj(wck_d, setB, lambda c: (K_sb[c], 0), knw_sb)
                v_proj(wcv_sb, setB, 0)

                # ctx half 1 reuses set A slots
                setC = [srcTp.tile([128, 4096], BF16, tag=f"sa{i}", bufs=1,
                                   name=f"cTb{i}") for i in range(4)]
                load_set(setC, cT_d[:, 1024:2048])
                wcv2_sb = load_wide(wcv_d, "c1")
                qk_proj(wck_d, setC, lambda c: (K_sb[c], 1024), knw_sb)
                v_proj(wcv2_sb, setC, TCH)

            # ================= attention + output phase =================
            with (
                tc.tile_pool(name="probsT", bufs=8) as probsp,
                tc.tile_pool(name="dacc", bufs=2) as daccp,
                tc.tile_pool(name="accb", bufs=2) as accbp,
                tc.tile_pool(name="attnTp", bufs=2) as attnTp,
                tc.tile_pool(name="wop", bufs=1) as wop,
                tc.tile_pool(name="atile", bufs=8) as atilep,
                tc.tile_pool(name="oacc", bufs=1) as oaccp,
                tc.tile_pool(name="mrow", bufs=4) as mrowp,
            ):
                wo_grp = [wop.tile([128, 4 * EW], BF16, tag=f"wo{i}", bufs=1,
                                   name=f"wo{i}") for i in range(4)]
                for i in range(4):
                    nc.sync.dma_start(
                        wo_grp[i][:].rearrange("p (a q) -> p a q", q=EW),
                        wo_d[i * 512:(i + 1) * 512, :]
                        .rearrange("(a p) q -> p a q", p=128))

                def wo_at(e):
                    return wo_grp[e // 4][:, (e % 4) * EW:(e % 4 + 1) * EW]
                out_acc = [oaccp.tile([128, EW], F32, tag=f"oa{t}", bufs=1,
                                      name=f"oa{t}") for t in range(TCH)]

                # attention for one local head, transposed scores:
                # scoresT[s-chunk] [128=s, 1024=q] = K_chunk @ Q^T  (no max
                # subtraction: scores ~ N(0,1) after rmsnorm + 1/sqrt(HD))
                def attention(h, q0=0, qw=K, dst=None):
                    # processes queries [q0, q0+qw) for local head h
                    nj = qw // 512
                    ps_pv = psA.tile([128, 1024], F32, tag="psA", name=f"pv{h}_{q0}")
                    acc = daccp.tile([128, 1024], F32, tag="dacc", name=f"dac{h}")
                    for s in range(SCH):
                        ps_sT = psA.tile([128, 1024], F32, tag="psA",
                                         name=f"sT{h}_{q0}_{s}")
                        for j in range(nj):
                            nc.tensor.matmul(
                                ps_sT[:, j * 512:(j + 1) * 512],
                                K_sb[h][:, s * 128:(s + 1) * 128],
                                QT_sb[h][:, q0 + j * 512:q0 + (j + 1) * 512],
                                start=True, stop=True)
                        if with_mask:
                            mrow = mrowp.tile([128, K], F32, tag="mrow")
                            nc.sync.dma_start(
                                mrow[:, :qw],
                                mt_d[s * 128:(s + 1) * 128, q0:q0 + qw])
                            nc.vector.tensor_tensor(ps_sT[:, :qw], ps_sT[:, :qw],
                                                    mrow[:, :qw], ALU.add)
                        pT = probsp.tile([128, 1024], BF16, tag="pT")
                        nc.scalar.activation(pT[:, :qw], ps_sT[:, :qw], AF.Exp,
                                             scale=SCALE)
                        first, last = (s == 0), (s == SCH - 1)
                        # probs accumulate on the DVE (f32) for the softmax
                        # denominators; the PE only does scores and PV
                        if first:
                            nc.vector.tensor_copy(acc[:, :qw], pT[:, :qw])
                        else:
                            nc.vector.tensor_tensor(acc[:, :qw], acc[:, :qw],
                                                    pT[:, :qw], ALU.add)
                        for j in range(nj):
                            js = slice(j * 512, (j + 1) * 512)
                            nc.tensor.matmul(
                                ps_pv[:, js],
                                V_sb[s][:, h * 128:(h + 1) * 128], pT[:, js],
                                start=first, stop=last)
                    # normalize: attnT = ps_pv * (1/denom), denom broadcast
                    # across partitions via K=1 matmul
                    at = attnTp.tile([128, K], BF16, tag="at")
                    accb = accbp.tile([128, 1024], BF16, tag="accb")
                    nc.vector.tensor_copy(accb[:, :qw], acc[:, :qw])
                    for j in range(nj):
                        js = slice(j * 512, (j + 1) * 512)
                        ps_d = ps1.tile([128, 512], F32, tag="ps1")
                        nc.tensor.matmul(ps_d[0:1, :], ones_col[:], accb[:, js],
                                         start=True, stop=True)
                        rec = statp.tile([1, 512], F32, tag="rec")
                        nc.vector.reciprocal(rec[:], ps_d[0:1, :])
                        rb = statp.tile([1, 512], BF16, tag="rb")
                        nc.vector.tensor_copy(rb[:], rec[:])
                        ps_b = ps1.tile([128, 512], F32, tag="ps1")
                        nc.tensor.matmul(ps_b[:], ones_row[:], rb[:],
                                         start=True, stop=True)
                        bc = bcp.tile([128, 512], F32, tag="bc")
                        nc.scalar.copy(bc[:], ps_b[:])
                        nc.vector.tensor_tensor(at[:, js], ps_pv[:, js], bc[:],
                                                ALU.mult)
                    dst_ap = dst if dst is not None else attnT_loc[h][:]
                    nc.sync.dma_start(dst_ap, at[:, :qw])

                # output projection wave for gathered head h:
                # out_acc[t] += attnT_gat[h][g-chunk, t-chunk].T @ wo[g*4+h]
                def out_wave(h, src=None, t0=0, tn=TCH):
                    gat = src if src is not None else attnT_gat[h]
                    for t in range(t0, t0 + tn):
                        ps = ps1.tile([128, 512], F32, tag="ps1")
                        at4 = atilep.tile([128, 512], BF16, tag="at")
                        nc.sync.dma_start(
                            at4[:].rearrange("p (g q) -> p g q", q=128),
                            gat[:, (t - t0) * 128:(t - t0 + 1) * 128]
                            .rearrange("(g p) q -> p g q", p=128))
                        for g in range(GROUPS):
                            nc.tensor.matmul(ps[:],
                                             at4[:, g * 128:(g + 1) * 128],
                                             wo_at(g * NH + h),
                                             start=(g == 0),
                                             stop=(g == GROUPS - 1))
                        if h == 0:
                            nc.vector.tensor_copy(out_acc[t][:], ps[:])
                        else:
                            nc.vector.tensor_tensor(out_acc[t][:], out_acc[t][:],
                                                    ps[:], ALU.add)

                # software pipeline: gather h overlaps attention h+1; the
                # out-projection wave for head h is emitted after attention
                # h+1 so the PE stream never waits on an in-flight gather.
                def gather(in_t, out_t):
                    nc.gpsimd.collective_compute(
                        "AllGather", ALU.bypass,
                        replica_groups=REPLICA_GROUPS,
                        ins=[in_t.opt()],
                        outs=[out_t.opt()],
                    )

                # attentions run back-to-back so every gather starts as
                # early as possible (the gather chain is the critical path);
                # the out-projection waves fill the tail while the last
                # gathers drain.
                for h in range(NH):
                    attention(h)
                    gather(attnT_loc[h], attnT_gat[h])
                for h in range(NH):
                    out_wave(h)

                for t in range(TCH):
                    nc.sync.dma_start(out_d[t * 128:(t + 1) * 128, :],
                                      out_acc[t][:])

    return nc


def _split_multiwaits(nc):
    """walrus codegen in this container rejects instructions with more than
    one semaphore wait; split the excess onto preceding NoOps on the same
    engine."""
    for f in nc.m.functions:
        for blk in f.blocks:
            idx = 0
            while idx < len(blk.instructions):
                inst = blk.instructions[idx]
                si = inst.sync_info
                maxw = 1
                if si is None or len(si.on_wait) <= maxw:
                    idx += 1
                    continue
                waits = list(si.on_wait)
                ncarry = (len(waits) - 1) // maxw  # leave <=maxw on inst
                for k in range(ncarry):
                    chunk = waits[k * maxw:(k + 1) * maxw]
                    nop = mybir.InstNoOp(
                        name=nc.get_next_instruction_name(),
                        ins=[], outs=[],
                        bass_nofuse=True,
                        sync_info=mybir.SyncInfo(on_wait=chunk, on_update=[]),
                    )
                    nop.engine = inst.engine
                    nc.register_instruction(nop)
                    blk.instructions.insert(idx, nop)
                    idx += 1
                si.on_wait = waits[ncarry * maxw:]
                idx += 1


def _get_program(with_mask: bool):
    key = ("prog", with_mask)
    if key not in _CACHE:
        nc = _build(with_mask)
        _split_multiwaits(nc)
        _CACHE[key] = nc
    return _CACHE[key]


def kernel(x, context, attn_mask, w_q, w_k, w_v, w_ctx_k, w_ctx_v, w_out,
           q_norm_w, k_norm_w):
    x = np.asarray(x, np.float32)
    context = np.asarray(context, np.float32)
    attn_mask = np.asarray(attn_mask, np.float32)
    w_q = np.asarray(w_q, np.float32)
    w_k = np.asarray(w_k, np.float32)
    w_v = np.asarray(w_v, np.float32)
    w_ctx_k = np.asarray(w_ctx_k, np.float32)
    w_ctx_v = np.asarray(w_ctx_v, np.float32)
    w_out = np.asarray(w_out, np.float32)
    q_norm_w = np.asarray(q_norm_w, np.float32)
    k_norm_w = np.asarray(k_norm_w, np.float32)

    with_mask = bool(np.any(attn_mask))
    nc = _get_program(with_mask)
    in_maps = _prepare_in_maps(x, context, attn_mask, w_q, w_k, w_v, w_ctx_k,
                               w_ctx_v, w_out, q_norm_w, k_norm_w, with_mask)

    res = run_bass_kernel_spmd(nc, in_maps, list(range(NCORES))).results
    return _assemble(res)


def _assemble(res):
    out = np.empty((B, K, D), np.float32)
    for c in range(NCORES):
        b, g = c // GROUPS, c % GROUPS
        out[b, :, g * EW:(g + 1) * EW] = res[c]["out"]
    return out


def _prepare_in_maps(x, context, attn_mask, w_q, w_k, w_v, w_ctx_k, w_ctx_v,
                     w_out, q_norm_w, k_norm_w, with_mask):
    bf16 = ml_dtypes.bfloat16
    xT = [np.ascontiguousarray(x[b].T).astype(bf16) for b in range(B)]
    cT = [np.ascontiguousarray(context[b].T).astype(bf16) for b in range(B)]
    in_maps = []
    for c in range(NCORES):
        b, g = c // GROUPS, c % GROUPS
        cols = slice(g * EW, (g + 1) * EW)
        m = {
            "xT": xT[b],
            "cT": cT[b],
            "wq": np.ascontiguousarray(w_q[:, cols]).astype(bf16),
            "wk": np.ascontiguousarray(w_k[:, cols]).astype(bf16),
            "wv": np.ascontiguousarray(w_v[:, cols]).astype(bf16),
            "wck": np.ascontiguousarray(w_ctx_k[:, cols]).astype(bf16),
            "wcv": np.ascontiguousarray(w_ctx_v[:, cols]).astype(bf16),
            "wo": np.ascontiguousarray(w_out[:, cols]).astype(bf16),
            "qnw": q_norm_w.reshape(HD, 1).astype(np.float32).copy(),
            "knw": k_norm_w.reshape(HD, 1).astype(np.float32).copy(),
        }
        if with_mask:
            # mask [B,1,K,S] -> transposed [S,K] per batch (fp32).
            # The kernel folds the 1/sqrt(HD) score scale into the exp
            # activation, which would scale the mask too; pre-divide so
            # exp((scores_raw + mask/SCALE) * SCALE) = exp(scores + mask).
            m["maskT"] = np.ascontiguousarray(attn_mask[b, 0].T) * (1.0 / SCALE)
        in_maps.append(m)
    return in_maps



# revision 10
# speedup vs baseline: 1.1358x; 1.1358x over previous
"""Trainium2 Bass kernel for DFlashAttentionV5.

Reference computation (fp32, single device):
    Q/K/V/Kctx/Vctx projections -> rmsnorm(Q), rmsnorm(K_full) -> softmax
    attention over concat(ctx, self) keys/values -> output projection.

Sharding over 8 NeuronCores: batch (2-way) x head-group (4-way).
Core c handles batch b = c // 4 and heads 4*g..4*g+3 where g = c % 4.

v2 structure (vs the AllGather baseline):
  * The output projection is computed locally at FULL width (each core
    multiplies its 4 heads' attention rows against its 512-row slice of
    w_out over all 2048 output columns) and a single ReduceScatter per
    query-quarter sums the partials across the 4-core batch group,
    writing each core's 512-column output slice directly.  This replaces
    four serialized 1MB AllGathers (4 x 41us) with four 256KB-out
    ReduceScatters (4 x 21.6us) that overlap the remaining compute.
  * Projections are reordered (ctx K/V, self K/V, then Q) so attention
    for head 0 starts as soon as its Q rows are normalized.
  * Softmax denominators accumulate in bf16 on the DVE (4x perf mode);
    cross-partition reductions and broadcasts for rmsnorm and the
    softmax denominator run on the otherwise-idle Pool engine
    (gpsimd partition_all_reduce / partition_broadcast), keeping the
    PE stream pure matmul.

All matmuls run in bf16 (fp32 PSUM accumulation); softmax statistics and
normalization factors are computed in fp32. x/ctx arrive pre-transposed
from the host ([D, tokens]) so no on-device transposes are needed.

Self-contained: hardcodes all shapes; only imports concourse + numpy.
"""

import math

import numpy as np
import ml_dtypes

import concourse.bass as bass
import concourse.bass_isa as bass_isa
import concourse.mybir as mybir
import concourse.tile as tile
from concourse.bass_utils import run_bass_kernel_spmd

BF16 = mybir.dt.bfloat16
F32 = mybir.dt.float32
AF = mybir.ActivationFunctionType
ALU = mybir.AluOpType
RED = bass_isa.ReduceOp

# Problem dims
B, K, CTX, D, H, HD = 2, 1024, 2048, 2048, 16, 128
S = CTX + K            # 3072 keys per query
NCORES = 8
GROUPS = 4             # head groups (tensor-parallel within a batch)
NH = H // GROUPS       # 4 local heads per core
E = H * HD             # 2048
EW = NH * HD           # 512 local attention width / weight shard width
DCH = D // 128         # 16 contraction chunks
SCH = S // 128         # 24 key chunks
TCH = K // 128         # 8 query-token chunks
NQ = 4                 # query quarters (out-proj + ReduceScatter granularity)
QW = K // NQ           # 256 queries per quarter
GB = 4                 # score chunks batched per exp activation (GB*QW = 1024)
NSG = SCH // GB        # 6 score groups per (head, quarter)
SCALE = 1.0 / math.sqrt(HD)
EPS = 1e-6
REPLICA_GROUPS = [[0, 1, 2, 3], [4, 5, 6, 7]]

_CACHE = {}


def _build(with_mask: bool):
    """Build the SPMD bass program (same program on all 8 cores)."""
    nc = bass.Bass(num_devices=NCORES)

    xT_d = nc.declare_dram_parameter("xT", [D, K], BF16, isOutput=False)
    cT_d = nc.declare_dram_parameter("cT", [D, CTX], BF16, isOutput=False)
    wq_d = nc.declare_dram_parameter("wq", [D, EW], BF16, isOutput=False)
    wk_d = nc.declare_dram_parameter("wk", [D, EW], BF16, isOutput=False)
    wv_d = nc.declare_dram_parameter("wv", [D, EW], BF16, isOutput=False)
    wck_d = nc.declare_dram_parameter("wck", [D, EW], BF16, isOutput=False)
    wcv_d = nc.declare_dram_parameter("wcv", [D, EW], BF16, isOutput=False)
    # w_out row-slice for this core's heads, FULL output width
    wo_d = nc.declare_dram_parameter("wo", [EW, E], BF16, isOutput=False)
    qnw_d = nc.declare_dram_parameter("qnw", [HD, 1], F32, isOutput=False)
    knw_d = nc.declare_dram_parameter("knw", [HD, 1], F32, isOutput=False)
    if with_mask:
        mt_d = nc.declare_dram_parameter("maskT", [S, K], F32, isOutput=False)
    out_d = nc.declare_dram_parameter("out", [K, EW], BF16, isOutput=True)

    with tile.TileContext(nc, num_cores=NCORES) as tc:
        with (
            tc.tile_pool(name="const", bufs=1) as constp,
            tc.tile_pool(name="perm", bufs=1) as perm,
            tc.tile_pool(name="stat", bufs=2) as statp,
            tc.tile_pool(name="bc", bufs=2) as bcp,
            tc.tile_pool(name="psA", bufs=2, space="PSUM") as psA,
            tc.tile_pool(name="ps1", bufs=2, space="PSUM") as ps1,
            tc.tile_pool(name="pv", bufs=2, space="PSUM") as pvp,
            tc.tile_pool(name="dram", bufs=1, space="DRAM") as dram,
        ):
            ones_col = constp.tile([128, 1], BF16)
            nc.any.memset(ones_col, 1.0)
            ones_row = constp.tile([1, 128], BF16)
            nc.any.memset(ones_row, 1.0)
            qnw_sb = constp.tile([HD, 1], F32)
            knw_sb = constp.tile([HD, 1], F32)

            # Resident tensors (bf16):
            #   K_sb[h]    [128=hd, 3072=s] per local head (ctx keys then self)
            #   V_sb[s]    [128=s-chunk, 512=4 heads x hd], s 0..15 ctx, 16..23 self
            #   QT_sb[h]   [128=hd, 1024=q]
            #   attnT_sb[h][128=hd, 1024=q] normalized attention rows
            K_sb = [perm.tile([128, S], BF16, tag=f"K{h}", bufs=1, name=f"K{h}")
                    for h in range(NH)]
            V_sb = [perm.tile([128, EW], BF16, tag=f"V{s}", bufs=1, name=f"V{s}")
                    for s in range(SCH)]
            QT_sb = [perm.tile([128, K], BF16, tag=f"Q{h}", bufs=1, name=f"Q{h}")
                     for h in range(NH)]
            attnT_sb = [perm.tile([128, K], BF16, tag=f"A{h}", bufs=1,
                                  name=f"A{h}") for h in range(NH)]

            po_dr = [dram.tile([GROUPS * QW, EW], BF16, name=f"po{i}")
                     for i in range(NQ)]
            # collectives cannot write IO tensors; RS lands here, then a
            # DMA moves each quarter to the output parameter
            rs_dr = [dram.tile([QW, EW], BF16, name=f"rs{i}")
                     for i in range(NQ)]

            # ---- helper: rmsnorm in transposed layout.
            # ps [128=hd, width=tokens] fp32 PSUM -> dest bf16 SBUF.
            # norm over hd (partitions): mean of squares via ones-matmul,
            # rsqrt via reciprocal+sqrt, broadcast across partitions via K=1
            # matmul, applied with one scalar_tensor_tensor (folds the norm
            # weight).
            def rms_norm_T(sqp, ps, dest_ap, width, nw_sb):
                sqt = sqp.tile([128, 1024], BF16, tag="sq")
                nc.scalar.square(sqt[:, :width], ps[:, :width])
                for j in range(width // 512):
                    js = slice(j * 512, (j + 1) * 512)
                    ps_s = ps1.tile([128, 512], F32, tag="ps1")
                    nc.tensor.matmul(ps_s[0:1, :], ones_col[:], sqt[:, js],
                                     start=True, stop=True)
                    mean = statp.tile([1, 512], F32, tag="mean")
                    nc.vector.tensor_scalar(mean[:], ps_s[0:1, :], 1.0 / HD,
                                            EPS, ALU.mult, ALU.add)
                    rec = statp.tile([1, 512], F32, tag="rec")
                    nc.vector.reciprocal(rec[:], mean[:])
                    rs = statp.tile([1, 512], BF16, tag="rs")
                    nc.scalar.sqrt(rs[:], rec[:])  # rsqrt = sqrt(1/x), bf16
                    ps_b = ps1.tile([128, 512], F32, tag="ps1")
                    nc.tensor.matmul(ps_b[:], ones_row[:], rs[:],
                                     start=True, stop=True)
                    bcv = bcp.tile([128, 512], F32, tag="bc")
                    nc.scalar.copy(bcv[:], ps_b[:])
                    nc.vector.scalar_tensor_tensor(
                        dest_ap[:, js], ps[:, js], nw_sb[:], bcv[:],
                        ALU.mult, ALU.mult)

            # ================= projection phase =================
            with (
                tc.tile_pool(name="srcT", bufs=1) as srcTp,
                tc.tile_pool(name="wstream", bufs=3) as wstream,
                tc.tile_pool(name="wwide", bufs=1) as wwide,
                tc.tile_pool(name="sqp", bufs=2) as sqp,
            ):
                # d-chunk accessor over grouped source tiles [128, 4*1024]
                def src_at(grp, d):
                    return grp[d // 4], (d % 4) * 1024

                def load_set(grp, dram_ap, split_first=False):
                    # dram_ap: [D, 1024] (d-major); one DMA per 4 d-chunks.
                    # split_first peels d-chunk 0 into its own small DMA so
                    # the first dependent matmul can start sooner.
                    if split_first:
                        nc.sync.dma_start(
                            grp[0][:, 0:1024],
                            dram_ap[0:128, :])
                        nc.sync.dma_start(
                            grp[0][:, 1024:4096].rearrange("p (a t) -> p a t",
                                                           t=1024),
                            dram_ap[128:512, :]
                            .rearrange("(a p) t -> p a t", p=128))
                    else:
                        nc.sync.dma_start(
                            grp[0][:].rearrange("p (a t) -> p a t", t=1024),
                            dram_ap[0:512, :]
                            .rearrange("(a p) t -> p a t", p=128))
                    for i in range(1, 4):
                        nc.sync.dma_start(
                            grp[i][:].rearrange("p (a t) -> p a t", t=1024),
                            dram_ap[i * 512:(i + 1) * 512, :]
                            .rearrange("(a p) t -> p a t", p=128))

                # Q^T / K^T projections (weight-stationary):
                # psum[c] [128=col-chunk(head), 1024 tokens] += w[d,c].T @ srcT[d]
                def load_wchunk(w_d, c, name=None):
                    wch = wstream.tile([128, D], BF16, tag="w", name=name)
                    nc.sync.dma_start(
                        wch[:].rearrange("p (a q) -> p a q", q=128),
                        w_d[:, c * 128:(c + 1) * 128]
                        .rearrange("(a p) q -> p a q", p=128))
                    return wch

                def qk_proj(w_d, srcT, dest_of_chunk, nw_sb, pre=None):
                    for c in range(EW // 128):
                        wch = pre if (c == 0 and pre is not None) \
                            else load_wchunk(w_d, c)
                        ps = psA.tile([128, 1024], F32, tag="psA",
                                      name=f"psqk{c}")
                        for d in range(DCH):
                            st, off = src_at(srcT, d)
                            for j in range(2):
                                nc.tensor.matmul(
                                    ps[:, j * 512:(j + 1) * 512],
                                    wch[:, d * 128:(d + 1) * 128],
                                    st[:, off + j * 512:off + j * 512 + 512],
                                    start=(d == 0), stop=(d == DCH - 1))
                        dest, off = dest_of_chunk(c)
                        rms_norm_T(sqp, ps, dest[:, off:off + 1024], 1024, nw_sb)

                # V projections (activation-stationary):
                # V_sb[s] [128=tokens, 512=cols] += srcT[d][:,t-chunk].T @ wv[d]
                def load_wide(w_d, tag):
                    grp = []
                    for i in range(4):
                        wt = wwide.tile([128, 4 * EW], BF16, tag=f"wv{i}",
                                        bufs=1, name=f"wv{i}_{tag}")
                        nc.sync.dma_start(
                            wt[:].rearrange("p (a q) -> p a q", q=EW),
                            w_d[i * 512:(i + 1) * 512, :]
                            .rearrange("(a p) q -> p a q", p=128))
                        grp.append(wt)
                    return grp

                def v_proj(wv_grp, srcT, s_base):
                    for t in range(TCH):
                        ps = ps1.tile([128, 512], F32, tag="ps1")
                        for d in range(DCH):
                            st, off = src_at(srcT, d)
                            wvt = wv_grp[d // 4]
                            wo_off = (d % 4) * EW
                            nc.tensor.matmul(
                                ps[:], st[:, off + t * 128:off + (t + 1) * 128],
                                wvt[:, wo_off:wo_off + EW],
                                start=(d == 0), stop=(d == DCH - 1))
                        if t % 2 == 0:
                            nc.vector.tensor_copy(V_sb[s_base + t][:], ps[:])
                        else:
                            nc.scalar.copy(V_sb[s_base + t][:], ps[:])

                # slot sets: A = ctx-half0 then xT (recycled), B = ctx-half1
                setA = [srcTp.tile([128, 4096], BF16, tag=f"sa{i}", bufs=1,
                                   name=f"cTa{i}") for i in range(4)]
                setB = [srcTp.tile([128, 4096], BF16, tag=f"sb{i}", bufs=1,
                                   name=f"cTb{i}") for i in range(4)]
                pre_wck = load_wchunk(wck_d, 0, name="prewck")
                load_set(setA, cT_d[:, 0:1024], split_first=True)
                nc.sync.dma_start(qnw_sb[:], qnw_d[:])
                nc.sync.dma_start(knw_sb[:], knw_d[:])

                # ctx half 0 (block A): K_ctx[:, 0:1024], V_ctx s-chunks 0..7
                wcv_sb = load_wide(wcv_d, "c0")
                qk_proj(wck_d, setA, lambda c: (K_sb[c], 0), knw_sb,
                        pre=pre_wck)
                load_set(setB, cT_d[:, 1024:2048])
                v_proj(wcv_sb, setA, 0)

                # ctx half 1 (block B): K_ctx[:, 1024:2048], V s-chunks 8..15
                wcv2_sb = load_wide(wcv_d, "c1")
                qk_proj(wck_d, setB, lambda c: (K_sb[c], 1024), knw_sb)
                v_proj(wcv2_sb, setB, TCH)

                # self tokens reuse set A slots: K_self, V_self, then Q last
                setC = [srcTp.tile([128, 4096], BF16, tag=f"sa{i}", bufs=1,
                                   name=f"xT{i}") for i in range(4)]
                load_set(setC, xT_d)
                wv_sb = load_wide(wv_d, "s")
                qk_proj(wk_d, setC, lambda c: (K_sb[c], CTX), knw_sb)
                v_proj(wv_sb, setC, CTX // 128)
                qk_proj(wq_d, setC, lambda c: (QT_sb[c], 0), qnw_sb)

            # ================= attention + output phase =================
            with (
                tc.tile_pool(name="probsT", bufs=4) as probsp,
                tc.tile_pool(name="acc", bufs=2) as accp,
                tc.tile_pool(name="wop", bufs=1) as wop,
                tc.tile_pool(name="outp", bufs=4) as outp,
                tc.tile_pool(name="mrow", bufs=2) as mrowp,
            ):
                # w_out rows for local head h: wo_loc[h] [128=hd, 2048=cols]
                wo_loc = [wop.tile([128, E], BF16, tag=f"wo{h}", bufs=1,
                                   name=f"wo{h}") for h in range(NH)]
                for h in range(NH):
                    nc.sync.dma_start(wo_loc[h][:],
                                      wo_d[h * 128:(h + 1) * 128, :])

                # attention for one (local head, query quarter), transposed
                # scores: scoresT[s-chunk] [128=s, QW=q] = K_chunk @ Q^T (no
                # max subtraction: scores ~ N(0,1) after rmsnorm + 1/sqrt(HD)).
                # GB score chunks share one [128, GB*QW] exp activation for
                # Act-engine efficiency; probs accumulate on the DVE in bf16
                # (4x perf mode) for the softmax denominators.
                def attention(h, iq):
                    q0 = iq * QW
                    acc = accp.tile([128, GB * QW], BF16, tag="acc",
                                    name=f"acc{h}_{iq}")
                    ps_pv = pvp.tile([128, QW], F32, tag="pv",
                                     name=f"pv{h}_{iq}")
                    for g in range(NSG):
                        ps4 = psA.tile([128, GB * QW], F32, tag="psA",
                                       name=f"sT{h}_{iq}_{g}")
                        for u in range(GB):
                            s = g * GB + u
                            nc.tensor.matmul(
                                ps4[:, u * QW:(u + 1) * QW],
                                K_sb[h][:, s * 128:(s + 1) * 128],
                                QT_sb[h][:, q0:q0 + QW],
                                start=True, stop=True)
                        if with_mask:
                            mrow = mrowp.tile([128, GB * QW], F32, tag="mrow")
                            nc.sync.dma_start(
                                mrow[:].rearrange("p (a q) -> p a q", q=QW),
                                mt_d[g * GB * 128:(g + 1) * GB * 128,
                                     q0:q0 + QW]
                                .rearrange("(a p) q -> p a q", p=128))
                            nc.vector.tensor_tensor(ps4[:], ps4[:], mrow[:],
                                                    ALU.add)
                        pT = probsp.tile([128, GB * QW], BF16, tag="pT")
                        nc.scalar.activation(pT[:], ps4[:], AF.Exp,
                                             scale=SCALE)
                        if g == 0:
                            nc.vector.tensor_copy(acc[:], pT[:])
                        else:
                            nc.vector.tensor_tensor(acc[:], acc[:], pT[:],
                                                    ALU.add)
                        for u in range(GB):
                            s = g * GB + u
                            nc.tensor.matmul(
                                ps_pv[:],
                                V_sb[s][:, h * 128:(h + 1) * 128],
                                pT[:, u * QW:(u + 1) * QW],
                                start=(g == 0 and u == 0),
                                stop=(g == NSG - 1 and u == GB - 1))
                    # softmax denominators: fold the GB accumulator columns,
                    # reduce across partitions via ones-matmul, reciprocal,
                    # broadcast back via K=1 matmul, normalize PV into the
                    # resident attnT rows.
                    facc = accp.tile([128, QW], BF16, tag="facc")
                    nc.vector.tensor_tensor(facc[:], acc[:, 0:QW],
                                            acc[:, QW:2 * QW], ALU.add)
                    nc.vector.tensor_tensor(facc[:], facc[:],
                                            acc[:, 2 * QW:3 * QW], ALU.add)
                    nc.vector.tensor_tensor(facc[:], facc[:],
                                            acc[:, 3 * QW:4 * QW], ALU.add)
                    ps_d = ps1.tile([128, 512], F32, tag="ps1")
                    nc.tensor.matmul(ps_d[0:1, :QW], ones_col[:], facc[:],
                                     start=True, stop=True)
                    rec = statp.tile([1, 512], F32, tag="rec")
                    nc.vector.reciprocal(rec[:, :QW], ps_d[0:1, :QW])
                    rb = statp.tile([1, 512], BF16, tag="rs")
                    nc.vector.tensor_copy(rb[:, :QW], rec[:, :QW])
                    ps_b = ps1.tile([128, 512], F32, tag="ps1")
                    nc.tensor.matmul(ps_b[:, :QW], ones_row[:], rb[:, :QW],
                                     start=True, stop=True)
                    bcv = bcp.tile([128, 512], F32, tag="bc")
                    nc.scalar.copy(bcv[:, :QW], ps_b[:, :QW])
                    nc.vector.tensor_tensor(attnT_sb[h][:, q0:q0 + QW],
                                            ps_pv[:], bcv[:, :QW], ALU.mult)

                # local output projection for one query quarter at FULL
                # output width from the 4 local heads, then ReduceScatter
                # sums partials across the batch group; group-rank r keeps
                # column block r, so the partial DRAM tile is laid out
                # column-block-major.
                def outproj(iq):
                    q0 = iq * QW
                    for t in range(QW // 128):
                        for j in range(GROUPS):
                            ps = ps1.tile([128, 512], F32, tag="ps1")
                            for h in range(NH):
                                nc.tensor.matmul(
                                    ps[:],
                                    attnT_sb[h][:, q0 + t * 128:
                                                q0 + (t + 1) * 128],
                                    wo_loc[h][:, j * 512:(j + 1) * 512],
                                    start=(h == 0), stop=(h == NH - 1))
                            osb = outp.tile([128, 512], BF16, tag="osb")
                            if (t * GROUPS + j) % 2 == 0:
                                nc.vector.tensor_copy(osb[:], ps[:])
                            else:
                                nc.scalar.copy(osb[:], ps[:])
                            nc.sync.dma_start(
                                po_dr[iq][j * QW + t * 128:
                                          j * QW + (t + 1) * 128, :],
                                osb[:])
                    nc.gpsimd.collective_compute(
                        "ReduceScatter", ALU.add,
                        replica_groups=REPLICA_GROUPS,
                        ins=[po_dr[iq][:].opt()],
                        outs=[rs_dr[iq][:].opt()],
                    )
                    nc.sync.dma_start(out_d[q0:q0 + QW, :], rs_dr[iq][:])

                for iq in range(NQ):
                    for h in range(NH):
                        attention(h, iq)
                    outproj(iq)

    return nc


def _split_multiwaits(nc):
    """walrus codegen in this container rejects instructions with more than
    one semaphore wait; split the excess onto preceding NoOps on the same
    engine."""
    for f in nc.m.functions:
        for blk in f.blocks:
            idx = 0
            while idx < len(blk.instructions):
                inst = blk.instructions[idx]
                si = inst.sync_info
                maxw = 1
                if si is None or len(si.on_wait) <= maxw:
                    idx += 1
                    continue
                waits = list(si.on_wait)
                ncarry = (len(waits) - 1) // maxw  # leave <=maxw on inst
                for k in range(ncarry):
                    chunk = waits[k * maxw:(k + 1) * maxw]
                    nop = mybir.InstNoOp(
                        name=nc.get_next_instruction_name(),
                        ins=[], outs=[],
                        bass_nofuse=True,
                        sync_info=mybir.SyncInfo(on_wait=chunk, on_update=[]),
                    )
                    nop.engine = inst.engine
                    nc.register_instruction(nop)
                    blk.instructions.insert(idx, nop)
                    idx += 1
                si.on_wait = waits[ncarry * maxw:]
                idx += 1


def _get_program(with_mask: bool):
    key = ("prog", with_mask)
    if key not in _CACHE:
        nc = _build(with_mask)
        _split_multiwaits(nc)
        _CACHE[key] = nc
    return _CACHE[key]


def kernel(x, context, attn_mask, w_q, w_k, w_v, w_ctx_k, w_ctx_v, w_out,
           q_norm_w, k_norm_w):
    x = np.asarray(x, np.float32)
    context = np.asarray(context, np.float32)
    attn_mask = np.asarray(attn_mask, np.float32)
    w_q = np.asarray(w_q, np.float32)
    w_k = np.asarray(w_k, np.float32)
    w_v = np.asarray(w_v, np.float32)
    w_ctx_k = np.asarray(w_ctx_k, np.float32)
    w_ctx_v = np.asarray(w_ctx_v, np.float32)
    w_out = np.asarray(w_out, np.float32)
    q_norm_w = np.asarray(q_norm_w, np.float32)
    k_norm_w = np.asarray(k_norm_w, np.float32)

    with_mask = bool(np.any(attn_mask))
    nc = _get_program(with_mask)
    in_maps = _prepare_in_maps(x, context, attn_mask, w_q, w_k, w_v, w_ctx_k,
                               w_ctx_v, w_out, q_norm_w, k_norm_w, with_mask)

    res = run_bass_kernel_spmd(nc, in_maps, list(range(NCORES))).results
    return _assemble(res)


def _assemble(res):
    out = np.empty((B, K, D), np.float32)
    for c in range(NCORES):
        b, g = c // GROUPS, c % GROUPS
        out[b, :, g * EW:(g + 1) * EW] = np.asarray(res[c]["out"], np.float32)
    return out


def _prepare_in_maps(x, context, attn_mask, w_q, w_k, w_v, w_ctx_k, w_ctx_v,
                     w_out, q_norm_w, k_norm_w, with_mask):
    bf16 = ml_dtypes.bfloat16
    xT = [np.ascontiguousarray(x[b].T).astype(bf16) for b in range(B)]
    cT = [np.ascontiguousarray(context[b].T).astype(bf16) for b in range(B)]
    in_maps = []
    for c in range(NCORES):
        b, g = c // GROUPS, c % GROUPS
        cols = slice(g * EW, (g + 1) * EW)
        m = {
            "xT": xT[b],
            "cT": cT[b],
            "wq": np.ascontiguousarray(w_q[:, cols]).astype(bf16),
            "wk": np.ascontiguousarray(w_k[:, cols]).astype(bf16),
            "wv": np.ascontiguousarray(w_v[:, cols]).astype(bf16),
            "wck": np.ascontiguousarray(w_ctx_k[:, cols]).astype(bf16),
            "wcv": np.ascontiguousarray(w_ctx_v[:, cols]).astype(bf16),
            # out-proj: this core's head ROWS at full output width
            "wo": np.ascontiguousarray(w_out[g * EW:(g + 1) * EW, :])
                  .astype(bf16),
            "qnw": q_norm_w.reshape(HD, 1).astype(np.float32).copy(),
            "knw": k_norm_w.reshape(HD, 1).astype(np.float32).copy(),
        }
        if with_mask:
            # mask [B,1,K,S] -> transposed [S,K] per batch (fp32).
            # The kernel folds the 1/sqrt(HD) score scale into the exp
            # activation, which would scale the mask too; pre-divide so
            # exp((scores_raw + mask/SCALE) * SCALE) = exp(scores + mask).
            m["maskT"] = np.ascontiguousarray(attn_mask[b, 0].T) * (1.0 / SCALE)
        in_maps.append(m)
    return in_maps
